# revision 1
# baseline (speedup 1.0000x reference)
"""Trainium2 Bass kernel for nn_CleanAttention (sliding-window GQA attention).

Problem: x[2,4096,2048] -> qkv proj -> rope -> sliding-window (256) attention
(16 q heads, 4 kv heads, d=128) -> o proj.

Sharding: 8 cores = batch(2) x token-quarters(4). Each core computes all 16
heads for its 1024 tokens, using a 256-token key/value halo on the left.
Outputs concatenate: no inter-core reduction.

Dataflow (per core), everything float32r on the PE (full rate, fp32 storage):
  xt  = x[b].T, zero-padded halo           [128p, 16kc, 1280t]
  V   = x @ wv.T   (natural layout)        10 x [128t, 512hd]
  K^T = wk_p @ x.T (+rope)                 4 x [128d, 1280t]
  Q^T = wq_p @ x.T (+rope)                 per (half,g): [128d, 4h, 512t]
  per chunk c (128 queries), kv-group g:
    S^T[kb]  = K^T_blk @ Q^T               3 x [128key, 512(h,q)]  (psum)
    S^T     += mask planes                 (DVE, in-place psum)
    E^T[kb]  = exp(scale * S^T)            (ACT -> f32r sbuf)
    sums     = ones128 @ E^T               [128, 512] all rows = colsums (psum)
    rec      = 1/sums                      (DVE)
    outT     = V_blk @ E^T                 [128d, 512(h,q)] (psum)
    yT       = outT * rec                  (DVE -> f32r sbuf)
  o = yT @ wo.T  accumulated over 16 heads -> [1024, 2048]

RoPE: wq/wk rows are host-permuted per head to [even dims | odd dims], so
rotation is 6 strided DVE ops on [64, T] halves with cos/sin lookup tiles.
"""

import math

import numpy as np

import concourse.bass as bass
import concourse.mybir as mybir
import concourse.tile as tile
from concourse import bacc
from concourse import bass_utils

B, T, C = 2, 4096, 2048
NH, NKV, D = 16, 4, 128
WINDOW = 256
N_CORES = 8
TCORE = 1024  # own tokens per core
HALO = 256
TX = TCORE + HALO  # 1280
NG = 4  # kv groups
GH = 4  # q heads per group
NCHUNK = 8  # query chunks of 128 per core
SCALE = 1.0 / math.sqrt(D)

f32 = mybir.dt.float32
f32r = mybir.dt.float32r

_CACHE = {}


def _build_nc(repeat=1, no_attn=False, no_oproj=False, no_rope=False, acc_bufs=5, soft_bufs=3, yt_bufs=16, mask_bufs=2, et_bufs=4, wsp_bufs=2, wop_bufs=4, raw_bufs=3, rt_bufs=4):
    nc = bacc.Bacc("TRN2", target_bir_lowering=False, debug=False)

    xt = nc.dram_tensor("xt", [128, 16, TX], f32r, kind="ExternalInput")
    wq_t = nc.dram_tensor("wq_t", [NH, 128, 16, 128], f32r, kind="ExternalInput")
    wk_t = nc.dram_tensor("wk_t", [NKV, 128, 16, 128], f32r, kind="ExternalInput")
    wv_t = nc.dram_tensor("wv_t", [128, 16, 512], f32r, kind="ExternalInput")
    wo_t = nc.dram_tensor("wo_t", [16, 4, 128, 512], f32r, kind="ExternalInput")
    cs_t = nc.dram_tensor("cs_t", [2, 128, TX], f32, kind="ExternalInput")
    masks = nc.dram_tensor("masks", [NCHUNK, 128, 2, 512], f32r, kind="ExternalInput")
    causal = nc.dram_tensor("causal", [128, 512], f32r, kind="ExternalInput")
    ones_in = nc.dram_tensor("ones_in", [128, 128], f32r, kind="ExternalInput")
    ident_in = nc.dram_tensor("ident_in", [128, 128], f32r, kind="ExternalInput")
    o_out = nc.dram_tensor("o_out", [TCORE, C], f32, kind="ExternalOutput")

    exp_t = mybir.ActivationFunctionType.Exp

    with tile.TileContext(nc) as tc:
        with (
            tc.sbuf_pool(name="fixed", bufs=1) as fixed,
            tc.sbuf_pool(name="ktp", bufs=1) as ktp,
            tc.sbuf_pool(name="vp", bufs=1) as vp,
            tc.sbuf_pool(name="ropetmp", bufs=rt_bufs) as ropetmp,
            tc.psum_pool(name="acc", bufs=acc_bufs) as acc,
            tc.psum_pool(name="soft", bufs=soft_bufs) as soft,
        ):
            # cos/sin lookup tiles, each duplicated on both partition halves so
            # DVE tensor_tensor base-partition constraints are satisfied
            cos_sb = fixed.tile([128, TX], f32)
            nc.sync.dma_start(cos_sb[:], cs_t[0])
            sin_sb = fixed.tile([128, TX], f32)
            nc.sync.dma_start(sin_sb[:], cs_t[1])
            ones_sb = fixed.tile([128, 128], f32r)
            nc.sync.dma_start(ones_sb[:], ones_in[:])
            causal_sb = fixed.tile([128, 512], f32r)
            nc.sync.dma_start(causal_sb[:], causal[:])
            ident_sb = fixed.tile([128, 128], f32r)
            nc.sync.dma_start(ident_sb[:], ident_in[:])

            def rope(dst_top, dst_bot, src_psum, col0, width):
                # Evacuate psum with one ACT copy so the PE isn't gated on
                # the (slow) DVE rotation chain, then rotate from SBUF.
                # src rows [0:64] = even dims, [64:128] = odd dims.
                # out_top = e*cos - o*sin ; out_bot = e*sin + o*cos
                raw = ropetmp.tile([128, 512], f32, name="raw", tag="raw", bufs=raw_bufs)
                nc.scalar.copy(raw[:, :width], src_psum[:])
                if no_rope:
                    nc.vector.tensor_copy(dst_top, raw[0:64, :width])
                    nc.vector.tensor_copy(dst_bot, raw[64:128, :width])
                    return
                sl = slice(col0, col0 + width)
                t1 = ropetmp.tile([64, 512], f32, name="t1", tag="rt")
                t2 = ropetmp.tile([64, 512], f32, name="t2", tag="rt")
                nc.vector.tensor_mul(t1[:, :width], raw[0:64, :width], cos_sb[0:64, sl])
                nc.vector.tensor_mul(t2[:, :width], raw[64:128, :width], sin_sb[64:128, sl])
                nc.vector.tensor_sub(dst_top, t1[:, :width], t2[:, :width])
                t3 = ropetmp.tile([64, 512], f32, name="t3", tag="rt")
                t4 = ropetmp.tile([64, 512], f32, name="t4", tag="rt")
                nc.vector.tensor_mul(t3[:, :width], raw[0:64, :width], sin_sb[0:64, sl])
                nc.vector.tensor_mul(t4[:, :width], raw[64:128, :width], cos_sb[64:128, sl])
                nc.vector.tensor_add(dst_bot, t3[:, :width], t4[:, :width])

            for rep in range(repeat):
                kt_tiles = [
                    ktp.tile([128, TX], f32r, name=f"ktg{g}", tag=f"ktg{g}")
                    for g in range(NG)
                ]
                v_tiles = [
                    vp.tile([128, 512], f32r, name=f"vtb{tb}", tag=f"vtb{tb}")
                    for tb in range(10)
                ]

                # ---- phase 0: V and K projections (x fully resident) ----
                with tc.sbuf_pool(name="xtf", bufs=1) as xtf:
                    xt_sb = xtf.tile([128, 16, TX], f32r)
                    nc.sync.dma_start(xt_sb[:], xt[:])

                    with tc.sbuf_pool(name="wvp", bufs=1) as wvp:
                        wv_sb = wvp.tile([128, 16, 512], f32r)
                        nc.sync.dma_start(wv_sb[:], wv_t[:])
                        for tb in range(10):
                            pv = acc.tile([128, 512], f32, name=f"pv{tb}", tag="acc")
                            for kc in range(16):
                                nc.tensor.matmul(
                                    pv[:],
                                    xt_sb[:, kc, tb * 128 : (tb + 1) * 128],
                                    wv_sb[:, kc, :],
                                    start=(kc == 0),
                                    stop=(kc == 15),
                                )
                            nc.scalar.copy(v_tiles[tb][:], pv[:])

                    with tc.sbuf_pool(name="wkp", bufs=2) as wkp:
                        for g in range(NG):
                            wk_sb = wkp.tile(
                                [128, 16, 128], f32r, name=f"wkg{g}", tag="wk"
                            )
                            nc.sync.dma_start(wk_sb[:], wk_t[g])
                            for ts, tw in [(0, 512), (512, 512), (1024, 256)]:
                                pk = acc.tile(
                                    [128, 512], f32, name=f"pk{g}_{ts}", tag="acc"
                                )
                                for kc in range(16):
                                    nc.tensor.matmul(
                                        pk[:, :tw],
                                        wk_sb[:, kc, :],
                                        xt_sb[:, kc, ts : ts + tw],
                                        start=(kc == 0),
                                        stop=(kc == 15),
                                    )
                                rope(
                                    kt_tiles[g][0:64, ts : ts + tw],
                                    kt_tiles[g][64:128, ts : ts + tw],
                                    pk[:, :tw],
                                    ts,
                                    tw,
                                )

                # ---- main: per half: Q proj + attention + O proj ----
                with (
                    tc.sbuf_pool(name="xth", bufs=1) as xthp,
                    tc.sbuf_pool(name="qtp", bufs=2) as qtp,
                    tc.sbuf_pool(name="ytp", bufs=yt_bufs) as ytp,
                    tc.sbuf_pool(name="wsp", bufs=wsp_bufs) as wsp,
                    tc.sbuf_pool(name="wop", bufs=wop_bufs) as wop,
                    tc.sbuf_pool(name="wrk", bufs=1) as wrk,
                ):
                    for half in range(2):
                        xth_sb = xthp.tile(
                            [128, 16, 512], f32r, name=f"xth{half}", tag="xth"
                        )
                        nc.sync.dma_start(
                            xth_sb[:],
                            xt[:, :, 256 + half * 512 : 256 + half * 512 + 512],
                        )
                        yts = {}
                        for g in range(NG):
                            qt_sb = qtp.tile(
                                [128, GH, 512], f32r, name=f"qt{half}_{g}", tag="qt"
                            )
                            for m in range(GH):
                                h = g * GH + m
                                wq_sb = wsp.tile(
                                    [128, 16, 128], f32r, name=f"wqh{h}", tag="wq"
                                )
                                nc.sync.dma_start(wq_sb[:], wq_t[h])
                                pq = acc.tile([128, 512], f32, name=f"pq{h}", tag="acc")
                                for kc in range(16):
                                    nc.tensor.matmul(
                                        pq[:],
                                        wq_sb[:, kc, :],
                                        xth_sb[:, kc, :],
                                        start=(kc == 0),
                                        stop=(kc == 15),
                                    )
                                rope(
                                    qt_sb[0:64, m, :],
                                    qt_sb[64:128, m, :],
                                    pq[:],
                                    256 + half * 512,
                                    512,
                                )
                            if no_attn:
                                for lc in range(4):
                                    c = half * 4 + lc
                                    yt = ytp.tile(
                                        [128, 512], f32r, name=f"yt{g}_{c}", tag="yt"
                                    )
                                    nc.scalar.copy(
                                        yt[:].rearrange("p (a b) -> p a b", a=4),
                                        qt_sb[:, :, lc * 128 : lc * 128 + 128],
                                    )
                                    yts[(g, lc)] = yt
                                continue
                            for lc in range(4):
                                c = half * 4 + lc
                                mask_sb = wrk.tile(
                                    [128, 2, 512], f32r, name=f"msk{c}", tag="mask",
                                    bufs=mask_bufs,
                                )
                                nc.sync.dma_start(mask_sb[:], masks[c])
                                sts = []
                                mask_planes = [
                                    mask_sb[:, 0, :],
                                    mask_sb[:, 1, :],
                                    causal_sb[:],
                                ]
                                for kb in range(3):
                                    st = soft.tile(
                                        [128, 512], f32, name=f"st{c}_{kb}", tag="soft"
                                    )
                                    # mask folded into the accumulation group:
                                    # st = I.T @ mask + K_blk @ Q^T
                                    nc.tensor.matmul(
                                        st[:],
                                        ident_sb[:],
                                        mask_planes[kb],
                                        start=True,
                                        stop=False,
                                    )
                                    nc.tensor.matmul(
                                        st[:],
                                        kt_tiles[g][
                                            :, c * 128 + kb * 128 : c * 128 + kb * 128 + 128
                                        ],
                                        qt_sb[:, :, lc * 128 : lc * 128 + 128],
                                        start=False,
                                        stop=True,
                                    )
                                    sts.append(st)
                                ets = []
                                for kb in range(3):
                                    et = wrk.tile(
                                        [128, 512], f32r, name=f"et{c}_{kb}", tag="et",
                                        bufs=et_bufs,
                                    )
                                    for hb in (0, 256):
                                        nc.scalar.activation(
                                            et[:, hb : hb + 256],
                                            sts[kb][:, hb : hb + 256],
                                            exp_t,
                                            bias=0.0,
                                            scale=SCALE,
                                        )
                                    ets.append(et)
                                sums = soft.tile(
                                    [128, 512], f32, name=f"sm{c}", tag="soft"
                                )
                                outt = soft.tile(
                                    [128, 512], f32, name=f"ot{c}", tag="soft"
                                )
                                for hb in (0, 256):
                                    for kb in range(3):
                                        nc.tensor.matmul(
                                            sums[:, hb : hb + 256],
                                            ones_sb[:],
                                            ets[kb][:, hb : hb + 256],
                                            start=(kb == 0),
                                            stop=(kb == 2),
                                        )
                                    for kb in range(3):
                                        nc.tensor.matmul(
                                            outt[:, hb : hb + 256],
                                            v_tiles[c + kb][:, g * 128 : (g + 1) * 128],
                                            ets[kb][:, hb : hb + 256],
                                            start=(kb == 0),
                                            stop=(kb == 2),
                                        )
                                rec = wrk.tile(
                                    [128, 512], f32, name=f"rc{c}", tag="rec", bufs=2
                                )
                                yt = ytp.tile(
                                    [128, 512], f32r, name=f"yt{g}_{c}", tag="yt"
                                )
                                for hb in (0, 256):
                                    nc.vector.reciprocal(
                                        rec[:, hb : hb + 256], sums[:, hb : hb + 256]
                                    )
                                    nc.vector.tensor_mul(
                                        yt[:, hb : hb + 256],
                                        outt[:, hb : hb + 256],
                                        rec[:, hb : hb + 256],
                                    )
                                yts[(g, lc)] = yt

                        # O projection for this half
                        if no_oproj:
                            for g in range(NG):
                                for lc in range(4):
                                    nc.sync.dma_start(
                                        o_out[
                                            (half * 4 + lc) * 128 : (half * 4 + lc)
                                            * 128
                                            + 128,
                                            g * 512 : g * 512 + 512,
                                        ],
                                        yts[(g, lc)][:].bitcast(f32),
                                    )
                            continue
                        for csx in range(4):
                            pos = [
                                acc.tile(
                                    [128, 512], f32,
                                    name=f"po{half}_{csx}_{lc}", tag="acc",
                                )
                                for lc in range(4)
                            ]
                            for mp in range(8):
                                wo_sb = wop.tile(
                                    [128, 2, 512], f32r, name=f"wo{mp}_{csx}", tag="wo"
                                )
                                nc.sync.dma_start(
                                    wo_sb[:],
                                    wo_t[2 * mp : 2 * mp + 2, csx].rearrange(
                                        "m p n -> p m n"
                                    ),
                                )
                                for mi in range(2):
                                    m = 2 * mp + mi
                                    for lc in range(4):
                                        nc.tensor.matmul(
                                            pos[lc][:],
                                            yts[(m // 4, lc)][
                                                :, (m % 4) * 128 : (m % 4) * 128 + 128
                                            ],
                                            wo_sb[:, mi, :],
                                            start=(m == 0),
                                            stop=(m == 15),
                                        )
                            for lc in range(4):
                                osb = wrk.tile(
                                    [128, 512], f32,
                                    name=f"osb{half}_{csx}_{lc}", tag="osb", bufs=2,
                                )
                                nc.scalar.copy(osb[:], pos[lc][:])
                                nc.sync.dma_start(
                                    o_out[
                                        (half * 4 + lc) * 128 : (half * 4 + lc) * 128
                                        + 128,
                                        csx * 512 : csx * 512 + 512,
                                    ],
                                    osb[:],
                                )

    nc.compile()
    return nc


def _prep_shared(wq, wk, wv, wo, rope_cache):
    """Host-side weight swizzles shared by all cores."""
    perm = np.concatenate([np.arange(0, 128, 2), np.arange(1, 128, 2)])

    wq_p = wq.reshape(NH, 128, C)[:, perm, :]  # [h, d, C]
    wq_sw = np.ascontiguousarray(
        wq_p.reshape(NH, 128, 16, 128).transpose(0, 3, 2, 1)
    ).astype(np.float32)  # [h, p, kc, n]

    wk_p = wk.reshape(NKV, 128, C)[:, perm, :]
    wk_sw = np.ascontiguousarray(
        wk_p.reshape(NKV, 128, 16, 128).transpose(0, 3, 2, 1)
    ).astype(np.float32)

    wv_sw = np.ascontiguousarray(
        wv.reshape(NKV * D, 16, 128).transpose(2, 1, 0)
    ).astype(np.float32)  # [p, kc, n=512]

    # wo given [C, HD]; need woT tiles [m, cs, p(d), n(c)]
    wo_sw = np.ascontiguousarray(
        wo.T.reshape(16, 128, 4, 512).transpose(0, 2, 1, 3)
    ).astype(np.float32)

    ones = np.ones((128, 128), dtype=np.float32)
    ident = np.eye(128, dtype=np.float32)

    # causal plane: key j, query i (replicated over 4 heads): valid j <= i
    j = np.arange(128)[:, None]
    i = np.arange(128)[None, :]
    causal = np.where(j <= i, 0.0, -1e30).astype(np.float32)
    causal4 = np.tile(causal, (1, 4))  # [128, 512]

    # interior mask planes (key-block kb0: valid j > i; kb1: all valid)
    kb0_int = np.where(j > i, 0.0, -1e30).astype(np.float32)
    kb0_int4 = np.tile(kb0_int, (1, 4))
    zeros4 = np.zeros((128, 512), dtype=np.float32)
    allinv4 = np.full((128, 512), -1e30, dtype=np.float32)

    return wq_sw, wk_sw, wv_sw, wo_sw, ones, ident, causal4, kb0_int4, zeros4, allinv4


def _make_in_maps(x, wq, wk, wv, wo, rope_cache):
    (wq_sw, wk_sw, wv_sw, wo_sw, ones, ident, causal4, kb0_int4, zeros4, allinv4) = (
        _prep_shared(wq, wk, wv, wo, rope_cache)
    )

    in_maps = []
    for core in range(N_CORES):
        b, tq = divmod(core, 4)
        t0 = tq * TCORE

        # x^T with left halo, zero-padded below t=0
        xpad = np.zeros((C, TX), dtype=np.float32)
        lo = t0 - HALO
        src_lo = max(lo, 0)
        xpad[:, src_lo - lo :] = x[b, src_lo : t0 + TCORE, :].T
        xt_sw = np.ascontiguousarray(xpad.reshape(16, 128, TX).transpose(1, 0, 2))

        # cos/sin tiles [2, 128, TX], each duplicated on both partition halves
        tglob = np.clip(np.arange(lo, t0 + TCORE), 0, T - 1)
        cs = np.empty((2, 128, TX), dtype=np.float32)
        cs[0, 0:64] = rope_cache[tglob, :, 0].T
        cs[0, 64:128] = cs[0, 0:64]
        cs[1, 0:64] = rope_cache[tglob, :, 1].T
        cs[1, 64:128] = cs[1, 0:64]

        # per-chunk mask planes [8, 128(key j), 2(kb), 512]
        mk = np.empty((NCHUNK, 128, 2, 512), dtype=np.float32)
        for c in range(NCHUNK):
            gc = t0 // 128 + c
            mk[c, :, 0, :] = kb0_int4 if gc >= 2 else allinv4
            mk[c, :, 1, :] = zeros4 if gc >= 1 else allinv4

        in_maps.append(
            {
                "xt": xt_sw,
                "wq_t": wq_sw,
                "wk_t": wk_sw,
                "wv_t": wv_sw,
                "wo_t": wo_sw,
                "cs_t": cs,
                "masks": mk,
                "causal": causal4,
                "ones_in": ones,
                "ident_in": ident,
            }
        )
    return in_maps


def kernel(x, wq, wk, wv, wo, rope_cache):
    x = np.asarray(x, dtype=np.float32)
    wq = np.asarray(wq, dtype=np.float32)
    wk = np.asarray(wk, dtype=np.float32)
    wv = np.asarray(wv, dtype=np.float32)
    wo = np.asarray(wo, dtype=np.float32)
    rope_cache = np.asarray(rope_cache, dtype=np.float32)

    if "nc" not in _CACHE:
        _CACHE["nc"] = _build_nc()
    nc = _CACHE["nc"]

    in_maps = _make_in_maps(x, wq, wk, wv, wo, rope_cache)
    _CACHE["in_maps"] = in_maps

    res = bass_utils.run_bass_kernel_spmd(nc, in_maps, core_ids=list(range(N_CORES)))

    out = np.empty((B, T, C), dtype=np.float32)
    for core in range(N_CORES):
        b, tq = divmod(core, 4)
        out[b, tq * TCORE : (tq + 1) * TCORE, :] = res.results[core]["o_out"]
    return out



# revision 38
# speedup vs baseline: 2.0240x; 2.0240x over previous
"""Trainium2 Bass kernel for nn_CleanAttention (sliding-window GQA attention).

Problem: x[2,4096,2048] -> qkv proj -> rope -> sliding-window (256) attention
(16 q heads, 4 kv heads, d=128) -> o proj.

Sharding: 8 cores = batch(2) x token-quarters(4). Each core computes all 16
heads for its 1024 tokens, using a 256-token key/value halo on the left.
Outputs concatenate: no inter-core reduction.

v2 dataflow (per core), all matmul operands bf16, psum f32:
  xt  = x[b].T bf16, zero-padded halo     [128p, 16kc, 1280t]  (resident)
  V   = x @ wv.T   (natural layout)       10 x [128t, 512hd] bf16
  K^T = wk_p @ x.T (+rope)                4 x [128d, 1280t] bf16
  Q^T = wq_p @ x.T (+rope)                per (half,g): [128d, 4h, 512t] bf16
  attention, software-pipelined one iter deep (i = (g,chunk)):
    S^T(i)   = K^T_blk @ Q^T  (3 psum banks, raw scores, no mask matmuls)
    E^T(i)   = exp(scale * S^T)           (ACT -> bf16 sbuf)
    E^T(i)  *= binmask (kb0 / causal)     (DVE bf16 2x, in sbuf)
    sums(i-1)= ones128 @ E^T              (psum)
    outT(i-1)= V_blk @ E^T                (psum)
    rec      = 1/(sums [+ corr])          (DVE; corr fixes chunk-0 halo rows)
    yT       = outT * rec                 (DVE -> bf16 sbuf)
  o = yT @ wo.T accumulated over 16 heads -> [1024, 2048] bf16 out

RoPE: wq/wk rows host-permuted per head to [even dims | odd dims]; rotation is
6 DVE ops at group width on bf16 (2x packed mode) with cos/sin lookup tiles.
Boundary masking: halo x is zero => K=0 => scores=0 => exp=1; binmasks zero the
kb0/causal planes, and the unmasked kb1 plane of chunk 0 is corrected by
subtracting 128 from the softmax denominator (corr plane, nonzero only on
cores 0 and 4).
"""

import math

import numpy as np
from ml_dtypes import bfloat16

import concourse.bass as bass
import concourse.mybir as mybir
import concourse.tile as tile
from concourse import bacc
from concourse import bass_utils

B, T, C = 2, 4096, 2048
NH, NKV, D = 16, 4, 128
WINDOW = 256
N_CORES = 8
TCORE = 1024  # own tokens per core
HALO = 256
TX = TCORE + HALO  # 1280
NG = 4  # kv groups
GH = 4  # q heads per group
NCHUNK = 8  # query chunks of 128 per core
SCALE = 1.0 / math.sqrt(D)

f32 = mybir.dt.float32
bf16 = mybir.dt.bfloat16

_CACHE = {}


def _build_nc(repeat=1, st_bufs=5, so_bufs=3, et_bufs=3, wq_bufs=4, wo_bufs=9,
              rt_bufs=3, rawq_bufs=2, yt_bufs=17, osb_bufs=4, skew=2):
    nc = bacc.Bacc("TRN2", target_bir_lowering=False, debug=False)

    xt = nc.dram_tensor("xt", [128, 16, TX], bf16, kind="ExternalInput")
    wq_t = nc.dram_tensor("wq_t", [NH, 128, 16, 128], bf16, kind="ExternalInput")
    wk_t = nc.dram_tensor("wk_t", [NKV, 128, 16, 128], bf16, kind="ExternalInput")
    wv_t = nc.dram_tensor("wv_t", [128, 16, 512], bf16, kind="ExternalInput")
    wo_t = nc.dram_tensor("wo_t", [16, 4, 128, 512], bf16, kind="ExternalInput")
    cs_t = nc.dram_tensor("cs_t", [2, 128, TX], bf16, kind="ExternalInput")
    csq_t = nc.dram_tensor("csq_t", [2, 128, 2, 4, 512], bf16, kind="ExternalInput")
    bmp_t = nc.dram_tensor("bmp_t", [128, 3, 2, 512], bf16, kind="ExternalInput")
    corr_t = nc.dram_tensor("corr_t", [128, 512], f32, kind="ExternalInput")
    ones_in = nc.dram_tensor("ones_in", [128, 128], bf16, kind="ExternalInput")
    o_out = nc.dram_tensor("o_out", [TCORE, C], bf16, kind="ExternalOutput")

    exp_t = mybir.ActivationFunctionType.Exp

    with tile.TileContext(nc) as tc:
        with (
            tc.sbuf_pool(name="fixed", bufs=1) as fixed,
            tc.sbuf_pool(name="xtp", bufs=1) as xtp,
            tc.sbuf_pool(name="ktp", bufs=1) as ktp,
            tc.sbuf_pool(name="vp", bufs=1) as vp,
            tc.sbuf_pool(name="ropetmp", bufs=1) as ropetmp,
            tc.sbuf_pool(name="wqp", bufs=wq_bufs) as wqp,
            tc.psum_pool(name="ps", bufs=st_bufs) as ps,
        ):
            # --- fixed small tables (order = DMA priority; compute-critical
            # loads for phase 0 are issued inside the rep loop before these
            # on the first pass via emission order) ---
            cos_sb = fixed.tile([128, TX], bf16)
            sin_sb = fixed.tile([128, TX], bf16)
            cosq_sb = fixed.tile([128, 2, 4, 512], bf16)
            sinq_sb = fixed.tile([128, 2, 4, 512], bf16)
            bmp_sb = fixed.tile([128, 3, 2, 512], bf16)
            corr_sb = fixed.tile([128, 512], f32)
            ones_sb = fixed.tile([128, 128], bf16)

            def load_fixed():
                nc.sync.dma_start(cos_sb[:], cs_t[0])
                nc.sync.dma_start(sin_sb[:], cs_t[1])
                nc.sync.dma_start(bmp_sb[:], bmp_t[:])
                nc.sync.dma_start(corr_sb[:], corr_t[:])
                nc.sync.dma_start(ones_sb[:], ones_in[:])
                nc.sync.dma_start(cosq_sb[:], csq_t[0])
                nc.sync.dma_start(sinq_sb[:], csq_t[1])

            def rope(dst, raw, c1_ap, c2_ap, width):
                # raw rows [0:64]=even dims e, [64:128]=odd dims o (bf16 sbuf)
                # c1 = [cos (top) | sin (bottom)], c2 = [sin | cos]
                # dst[0:64] = e*cos - o*sin ; dst[64:128] = e*sin + o*cos
                # (tensor_tensor inputs must share the start partition, so
                # each product lands in a base-0 temp tile first)
                t1 = ropetmp.tile([64, 2048], bf16, name="t1", tag="rt",
                                  bufs=rt_bufs)
                t2 = ropetmp.tile([64, 2048], bf16, name="t2", tag="rt",
                                  bufs=rt_bufs)
                nc.vector.tensor_mul(t1[:, :width], raw[0:64], c1_ap[0:64])
                nc.vector.tensor_mul(t2[:, :width], raw[64:128], c1_ap[64:128])
                nc.vector.tensor_sub(dst[0:64], t1[:, :width], t2[:, :width])
                t3 = ropetmp.tile([64, 2048], bf16, name="t3", tag="rt",
                                  bufs=rt_bufs)
                t4 = ropetmp.tile([64, 2048], bf16, name="t4", tag="rt",
                                  bufs=rt_bufs)
                nc.vector.tensor_mul(t3[:, :width], raw[0:64], c2_ap[0:64])
                nc.vector.tensor_mul(t4[:, :width], raw[64:128], c2_ap[64:128])
                nc.vector.tensor_add(dst[64:128], t3[:, :width], t4[:, :width])

            for rep in range(repeat):
                xt_sb = xtp.tile([128, 16, TX], bf16, name="xt_sb", tag="xt")
                kt_tiles = [
                    ktp.tile([128, TX], bf16, name=f"ktg{g}", tag=f"ktg{g}")
                    for g in range(NG)
                ]
                v_tiles = [
                    vp.tile([128, 512], bf16, name=f"vtb{tb}", tag=f"vtb{tb}")
                    for tb in range(10)
                ]

                # ---- phase 0: V and K projections ----
                with (
                    tc.sbuf_pool(name="wvp", bufs=1) as wvp,
                    tc.sbuf_pool(name="wkp", bufs=2) as wkp,
                    tc.sbuf_pool(name="krawp", bufs=4) as krawp,
                ):
                    # DMA priority order: wv + first x chunks feed the first
                    # matmuls; tables and Q-side constants come later.
                    wv_sb = wvp.tile([128, 16, 512], bf16)
                    xchunks = [(512, 256), (768, 256), (1024, 256)]
                    nc.sync.dma_start(xt_sb[:, :, 0:128], xt[:, :, 0:128])
                    for kc2 in range(8):
                        nc.sync.dma_start(wv_sb[:, 2 * kc2 : 2 * kc2 + 2, :],
                                          wv_t[:, 2 * kc2 : 2 * kc2 + 2, :])
                    nc.sync.dma_start(xt_sb[:, :, 128:512], xt[:, :, 128:512])
                    wk_sbs = []
                    for g in range(NG):
                        wk_sb = wkp.tile([128, 16, 128], bf16, name=f"wkg{g}",
                                         tag="wk", bufs=NG)
                        nc.sync.dma_start(wk_sb[:], wk_t[g])
                        wk_sbs.append(wk_sb)
                    for ts, tw in xchunks:
                        nc.sync.dma_start(xt_sb[:, :, ts : ts + tw],
                                          xt[:, :, ts : ts + tw])
                    if rep == 0:
                        load_fixed()
                    del xchunks

                    def vproj(tb):
                        pv = ps.tile([128, 512], f32, name=f"pv{tb}", tag="ps")
                        for kc in range(16):
                            nc.tensor.matmul(
                                pv[:],
                                xt_sb[:, kc, tb * 128 : (tb + 1) * 128],
                                wv_sb[:, kc, :],
                                start=(kc == 0),
                                stop=(kc == 15),
                            )
                        nc.scalar.copy(v_tiles[tb][:], pv[:])

                    kraws = {}

                    def kproj(g, ts, tw):
                        pk = ps.tile([128, 512], f32, name=f"pk{g}_{ts}", tag="ps")
                        for kc in range(16):
                            nc.tensor.matmul(
                                pk[:, :tw],
                                wk_sbs[g][:, kc, :],
                                xt_sb[:, kc, ts : ts + tw],
                                start=(kc == 0),
                                stop=(kc == 15),
                            )
                        if g not in kraws:
                            kraws[g] = krawp.tile([128, TX], bf16,
                                                  name=f"kraw{g}", tag="kraw")
                        nc.scalar.copy(kraws[g][:, ts : ts + tw], pk[:, :tw])

                    # interleave with xt chunk arrival order
                    for tb in (0, 1, 2, 3):
                        vproj(tb)
                    for g in range(NG):
                        kproj(g, 0, 512)
                    for tb in (4, 5, 6, 7):
                        vproj(tb)
                    for g in range(NG):
                        kproj(g, 512, 512)
                    for tb in (8, 9):
                        vproj(tb)
                    for g in range(NG):
                        kproj(g, 1024, 256)
                        rope(kt_tiles[g], kraws[g], cos_sb, sin_sb, TX)

                # ---- main: per half: Q proj + attention + O proj ----
                with (
                    tc.sbuf_pool(name="qtp", bufs=5) as qtp,
                    tc.sbuf_pool(name="ytp", bufs=yt_bufs) as ytp,
                    tc.sbuf_pool(name="wop", bufs=wo_bufs) as wop,
                    tc.sbuf_pool(name="wrk", bufs=1) as wrk,
                ):
                    def oproj_fn(hh, hyts):
                        def o_evac(csx, lc, pos_lc):
                            osb = wrk.tile([128, 512], bf16,
                                           name=f"osb{hh}_{csx}_{lc}",
                                           tag="osb", bufs=osb_bufs)
                            nc.scalar.copy(osb[:], pos_lc[:])
                            nc.sync.dma_start(
                                o_out[
                                    (hh * 4 + lc) * 128 : (hh * 4 + lc)
                                    * 128 + 128,
                                    csx * 512 : csx * 512 + 512,
                                ],
                                osb[:],
                            )

                        for csx in range(4):
                            last = hh == 1 and csx == 3
                            pos = [
                                ps.tile([128, 512], f32,
                                        name=f"po{hh}_{csx}_{lc}", tag="ps")
                                for lc in range(4)
                            ]
                            wo_sbs = []
                            for mp in range(8):
                                wo_sb = wop.tile([128, 2, 512], bf16,
                                                 name=f"wo{hh}_{mp}_{csx}",
                                                 tag="wo")
                                nc.sync.dma_start(
                                    wo_sb[:],
                                    wo_t[2 * mp : 2 * mp + 2, csx].rearrange(
                                        "m p n -> p m n"
                                    ),
                                )
                                wo_sbs.append(wo_sb)
                            mm_order = (
                                [(lc, mp, mi) for lc in range(4)
                                 for mp in range(8) for mi in range(2)]
                                if last else
                                [(lc, mp, mi) for mp in range(8)
                                 for mi in range(2) for lc in range(4)]
                            )
                            for lc, mp, mi in mm_order:
                                m = 2 * mp + mi
                                nc.tensor.matmul(
                                    pos[lc][:],
                                    hyts[(m // 4, lc)][
                                        :, (m % 4) * 128 : (m % 4) * 128 + 128
                                    ],
                                    wo_sbs[mp][:, mi, :],
                                    start=(m == 0),
                                    stop=(m == 15),
                                )
                                if last and m == 15:
                                    o_evac(csx, lc, pos[lc])
                            if not last:
                                for lc in range(4):
                                    o_evac(csx, lc, pos[lc])

                    prev_oproj = None
                    for half in range(2):
                        tok0 = 256 + half * 512  # local token offset of queries
                        iters = [(g, lc) for g in range(NG) for lc in range(4)]
                        state = {}  # i -> (et, c, g)
                        qts = []
                        yts = {}

                        def qproj(g):
                            qt_sb = qtp.tile([128, GH, 512], bf16,
                                             name=f"qt{half}_{g}", tag="qt")
                            qraw = ropetmp.tile([128, 4, 512], bf16,
                                                name=f"qraw{g}", tag="qraw",
                                                bufs=rawq_bufs)
                            for m in range(GH):
                                h = g * GH + m
                                wq_sb = wqp.tile([128, 16, 128], bf16,
                                                 name=f"wqh{h}", tag="wq")
                                nc.sync.dma_start(wq_sb[:], wq_t[h])
                                pq = ps.tile([128, 512], f32, name=f"pq{h}",
                                             tag="ps")
                                for kc in range(16):
                                    nc.tensor.matmul(
                                        pq[:],
                                        wq_sb[:, kc, :],
                                        xt_sb[:, kc, tok0 : tok0 + 512],
                                        start=(kc == 0),
                                        stop=(kc == 15),
                                    )
                                nc.scalar.copy(qraw[:, m, :], pq[:])
                            rope(qt_sb[:].rearrange("p a b -> p (a b)"),
                                 qraw[:].rearrange("p a b -> p (a b)"),
                                 cosq_sb[:, half].rearrange("p a b -> p (a b)"),
                                 sinq_sb[:, half].rearrange("p a b -> p (a b)"),
                                 2048)
                            qts.append(qt_sb)

                        # attention emission, software-pipelined `skew` deep.
                        # et planes are (kb0, causal, kb1) in one tile so both
                        # binmasks apply in a single DVE multiply.
                        PLANES = (0, 2, 1)  # plane j holds key-block PLANES[j]

                        def emit_scores(i):
                            g, lc = iters[i]
                            c = half * 4 + lc
                            et = wrk.tile([128, 3, 512], bf16,
                                          name=f"et{c}_{g}", tag="et",
                                          bufs=et_bufs)
                            for j, kb in enumerate(PLANES):
                                st = ps.tile([128, 512], f32,
                                             name=f"st{c}_{g}_{kb}", tag="ps")
                                nc.tensor.matmul(
                                    st[:],
                                    kt_tiles[g][:, c * 128 + kb * 128 :
                                                c * 128 + kb * 128 + 128],
                                    qts[g][:, :, lc * 128 : lc * 128 + 128],
                                    start=True,
                                    stop=True,
                                )
                                nc.scalar.activation(et[:, j, :], st[:],
                                                     exp_t, bias=0.0, scale=SCALE)
                            state[i] = (et, c, g)

                        def emit_masks(i):
                            # kb0 plane on (idle) Pool; causal on Pool early
                            # (while DVE chews the g3 rope), DVE once it's free
                            et, c, g = state[i]
                            v = min(c, 2)
                            nc.gpsimd.tensor_mul(et[:, 0, :], et[:, 0, :],
                                                 bmp_sb[:, v, 0])
                            eng = nc.gpsimd if i < 4 else nc.vector
                            eng.tensor_mul(et[:, 1, :], et[:, 1, :],
                                           bmp_sb[:, v, 1])

                        def emit_reduce(i):
                            et, c, g = state.pop(i)
                            sums = ps.tile([128, 512], f32, name=f"sm{c}_{g}",
                                           tag="so", bufs=so_bufs)
                            outt = ps.tile([128, 512], f32, name=f"ot{c}_{g}",
                                           tag="so", bufs=so_bufs)
                            # consume the (DVE-masked) causal plane 1 last
                            for n, j in enumerate((0, 2, 1)):
                                nc.tensor.matmul(
                                    sums[:], ones_sb[:], et[:, j, :],
                                    start=(n == 0), stop=(n == 2),
                                )
                            for n, j in enumerate((0, 2, 1)):
                                kb = PLANES[j]
                                nc.tensor.matmul(
                                    outt[:],
                                    v_tiles[c + kb][:, g * 128 : (g + 1) * 128],
                                    et[:, j, :],
                                    start=(n == 0), stop=(n == 2),
                                )
                            rec = wrk.tile([128, 512], f32, name=f"rc{c}_{g}",
                                           tag="rec", bufs=2)
                            if c == 0:
                                sadj = wrk.tile([128, 512], f32,
                                                name=f"sa{g}", tag="sadj", bufs=2)
                                nc.vector.tensor_add(sadj[:], sums[:], corr_sb[:])
                                nc.vector.reciprocal_approx_fast(rec[:], sadj[:])
                            else:
                                nc.vector.reciprocal_approx_fast(rec[:], sums[:])
                            yt = ytp.tile([128, 512], bf16, name=f"yt{g}_{c}",
                                          tag="yt")
                            nc.vector.tensor_mul(yt[:], outt[:], rec[:])
                            yts[(g, c % 4)] = yt

                        # Q proj g0..g2, then warmup scores so exp latency
                        # hides under the g3 projection matmuls; the previous
                        # half's O projection then fills the pipeline further.
                        qproj(0)
                        qproj(1)
                        qproj(2)
                        emit_scores(0)
                        emit_masks(0)
                        emit_scores(1)
                        emit_masks(1)
                        qproj(3)
                        if prev_oproj is not None:
                            prev_oproj()
                            prev_oproj = None
                        for i in range(skew, 16):
                            emit_scores(i)
                            emit_reduce(i - skew)
                            emit_masks(i)
                        for i in range(16 - skew, 16):
                            emit_reduce(i)

                        # -- O projection: half 0's is deferred into half 1's
                        # warmup; half 1's runs at the end --
                        if half == 0:
                            prev_oproj = (lambda hh=half, hy=yts:
                                          oproj_fn(hh, hy))
                        else:
                            oproj_fn(half, yts)

    nc.compile()
    return nc


def _prep_shared(wq, wk, wv, wo, rope_cache):
    """Host-side weight swizzles shared by all cores."""
    perm = np.concatenate([np.arange(0, 128, 2), np.arange(1, 128, 2)])

    wq_p = wq.reshape(NH, 128, C)[:, perm, :]  # [h, d, C]
    wq_sw = np.ascontiguousarray(
        wq_p.reshape(NH, 128, 16, 128).transpose(0, 3, 2, 1)
    ).astype(bfloat16)  # [h, p, kc, n]

    wk_p = wk.reshape(NKV, 128, C)[:, perm, :]
    wk_sw = np.ascontiguousarray(
        wk_p.reshape(NKV, 128, 16, 128).transpose(0, 3, 2, 1)
    ).astype(bfloat16)

    wv_sw = np.ascontiguousarray(
        wv.reshape(NKV * D, 16, 128).transpose(2, 1, 0)
    ).astype(bfloat16)  # [p, kc, n=512]

    # wo given [C, HD]; need woT tiles [m, cs, p(d), n(c)]
    wo_sw = np.ascontiguousarray(
        wo.T.reshape(16, 128, 4, 512).transpose(0, 2, 1, 3)
    ).astype(bfloat16)

    ones = np.ones((128, 128), dtype=bfloat16)

    # binary masks: key j (partition), query i (free, replicated over 4 heads)
    j = np.arange(128)[:, None]
    i = np.arange(128)[None, :]
    tri_kb0 = np.tile((j > i).astype(np.float32), (1, 4)).astype(bfloat16)
    tri_cau = np.tile((j <= i).astype(np.float32), (1, 4)).astype(bfloat16)
    zeros4 = np.zeros((128, 512), dtype=bfloat16)

    return wq_sw, wk_sw, wv_sw, wo_sw, ones, tri_kb0, tri_cau, zeros4


def _make_in_maps(x, wq, wk, wv, wo, rope_cache):
    (wq_sw, wk_sw, wv_sw, wo_sw, ones, tri_kb0, tri_cau, zeros4) = _prep_shared(
        wq, wk, wv, wo, rope_cache
    )

    in_maps = []
    for core in range(N_CORES):
        b, tq = divmod(core, 4)
        t0 = tq * TCORE
        boundary = t0 == 0

        # x^T with left halo, zero-padded below t=0
        xpad = np.zeros((C, TX), dtype=np.float32)
        lo = t0 - HALO
        src_lo = max(lo, 0)
        xpad[:, src_lo - lo :] = x[b, src_lo : t0 + TCORE, :].T
        xt_sw = np.ascontiguousarray(
            xpad.reshape(16, 128, TX).transpose(1, 0, 2)
        ).astype(bfloat16)

        # combined rope tables: c1 = [cos | sin], c2 = [sin | cos] stacked on
        # partition halves (matching raw's [even | odd] layout)
        tglob = np.clip(np.arange(lo, t0 + TCORE), 0, T - 1)
        cosv = rope_cache[tglob, :, 0].T  # [64, TX]
        sinv = rope_cache[tglob, :, 1].T
        cs = np.empty((2, 128, TX), dtype=np.float32)
        cs[0, 0:64] = cosv
        cs[0, 64:128] = sinv
        cs[1, 0:64] = sinv
        cs[1, 64:128] = cosv

        # Q-side tables repeated over the 4 heads of a group: [2,128,2,4,512]
        csq = np.empty((2, 128, 2, 4, 512), dtype=np.float32)
        for half in range(2):
            sl = slice(256 + half * 512, 256 + half * 512 + 512)
            csq[0, :, half] = np.broadcast_to(cs[0, :, sl][:, None, :],
                                              (128, 4, 512))
            csq[1, :, half] = np.broadcast_to(cs[1, :, sl][:, None, :],
                                              (128, 4, 512))

        # binmask plane pairs (kb0, causal) with kb0 variant by min(chunk, 2)
        bmp = np.empty((128, 3, 2, 512), dtype=bfloat16)
        for v in range(3):
            bmp[:, v, 0] = zeros4 if (boundary and v < 2) else tri_kb0
            bmp[:, v, 1] = tri_cau

        corr = np.full((128, 512), -128.0 if boundary else 0.0, dtype=np.float32)

        in_maps.append(
            {
                "xt": xt_sw,
                "wq_t": wq_sw,
                "wk_t": wk_sw,
                "wv_t": wv_sw,
                "wo_t": wo_sw,
                "cs_t": cs.astype(bfloat16),
                "csq_t": csq.astype(bfloat16),
                "bmp_t": bmp,
                "corr_t": corr,
                "ones_in": ones,
            }
        )
    return in_maps


def kernel(x, wq, wk, wv, wo, rope_cache):
    x = np.asarray(x, dtype=np.float32)
    wq = np.asarray(wq, dtype=np.float32)
    wk = np.asarray(wk, dtype=np.float32)
    wv = np.asarray(wv, dtype=np.float32)
    wo = np.asarray(wo, dtype=np.float32)
    rope_cache = np.asarray(rope_cache, dtype=np.float32)

    if "nc" not in _CACHE:
        _CACHE["nc"] = _build_nc()
    nc = _CACHE["nc"]

    in_maps = _make_in_maps(x, wq, wk, wv, wo, rope_cache)
    _CACHE["in_maps"] = in_maps

    res = bass_utils.run_bass_kernel_spmd(nc, in_maps, core_ids=list(range(N_CORES)))

    out = np.empty((B, T, C), dtype=np.float32)
    for core in range(N_CORES):
        b, tq = divmod(core, 4)
        out[b, tq * TCORE : (tq + 1) * TCORE, :] = res.results[core][
            "o_out"
        ].astype(np.float32)
    return out
